# revision 16
# baseline (speedup 1.0000x reference)
"""GCN-VAE forward pass (GCNModelVAE) on 8 Trainium2 NeuronCores.

Row-shards the 8192 nodes across 8 cores (1024 rows each). All big matmuls
run in bf16 on the PE array with fp32 PSUM accumulation; the output is
saturation-dominated (the reference's exp(log_std) overflows), so bf16
operand precision is ample.

Per core (M = 1024 local nodes, P = 128 partitions):
  0. Cast adj_s fp32 -> bf16 as two row-half DRAM tensors (row-contiguous
     SWDGE casts are bandwidth-bound; column-chunked ones are
     descriptor-bound).  x/eps cast contiguously; W1/Wms cast via DVE.
  A. xW1_s = x_s @ W1 via transposed x tiles; AllGather -> xW1_full.
  B. hT_s = relu(xW1_full^T @ adj_s^T): adj^T [128,512] tiles via DMA-xbar
     transpose loads (sync queue), one per (kt, row-half); the transposed
     tiles are also stored to adjT in DRAM (scalar queue) for pass 2.
  C. hWms_s = h_s @ [Wm|Ws] [1024,256] locally; AllGather -> hWms_full.
  D. zmT/lsT = (hWms)^T @ adj_s^T via plain reloads of adjT (no xbar).
  E. zT = zmT + epsT * exp(lsT); AllGather zT -> z_all.
  F. out_s = sigmoid(zT_s^T @ zT_all) [1024,8192] row-block of the decoder.

Queue discipline (avoids HWDGE/SWDGE head-of-line stalls found in traces):
  gpsimd/SWDGE q0: contiguous casts + half the decoder stores
  sync  HWDGE: all xbar transposes, pass-2 loads, half the decoder stores
  scalar HWDGE: bounce stores, gathered loads, adjT stores
"""

import numpy as np

import concourse.bacc as bacc
import concourse.mybir as mybir
import concourse.tile as tile
from concourse.bass_utils import run_bass_kernel_spmd

N = 8192
F_IN = 512
H1 = 256
H2 = 128
NC = 8
M = N // NC          # 1024 rows per core
P = 128
KT = N // P          # 64 k-tiles over the node dimension
MT = M // P          # 8 m-tiles per core
HM = M // 2          # 512 = row-half size
F32 = mybir.dt.float32
BF16 = mybir.dt.bfloat16
AF = mybir.ActivationFunctionType


def build(n_cores=NC):
    nc = bacc.Bacc("TRN2", target_bir_lowering=False, debug=False,
                   num_devices=n_cores)
    x_s = nc.dram_tensor("x_s", [M, F_IN], F32, kind="ExternalInput")
    adj_s = nc.dram_tensor("adj_s", [M, N], F32, kind="ExternalInput")
    W1 = nc.dram_tensor("W1", [F_IN, H1], F32, kind="ExternalInput")
    Wm = nc.dram_tensor("Wm", [H1, H2], F32, kind="ExternalInput")
    Ws = nc.dram_tensor("Ws", [H1, H2], F32, kind="ExternalInput")
    eps_s = nc.dram_tensor("eps_s", [M, H2], F32, kind="ExternalInput")
    out_s = nc.dram_tensor("out_s", [M, N], F32, kind="ExternalOutput")

    rg = [list(range(n_cores))]

    with tile.TileContext(nc) as tc:
        with (
            tc.tile_pool(name="pers", bufs=1) as pers,
            tc.tile_pool(name="mv", bufs=1) as mv,
            tc.tile_pool(name="ev", bufs=6) as ev,
        ):
            # ---------------- DRAM staging (raw internal tensors) --------
            adj_bh = [nc.dram_tensor(f"adj_bh{h}", [HM, N], BF16).ap()
                      for h in range(2)]
            adjT = nc.dram_tensor("adjT", [N, M], BF16).ap()
            x_b = nc.dram_tensor("x_b", [M, F_IN], BF16).ap()
            eps_b = nc.dram_tensor("eps_b", [M, H2], BF16).ap()
            warm_in = nc.dram_tensor("warm_in", [P, 1], BF16).ap()
            warm_out = nc.dram_tensor("warm_out", [P * NC, 1], BF16,
                                      addr_space="Shared").ap()
            xw1_bounce = nc.dram_tensor("xw1_bounce", [M, H1], BF16).ap()
            xw1_all = nc.dram_tensor("xw1_all", [N, H1], BF16,
                                     addr_space="Shared").ap()
            hwms_bounce = nc.dram_tensor("hwms_bounce", [M, 2 * H2], BF16).ap()
            hwms_all = nc.dram_tensor("hwms_all", [N, 2 * H2], BF16,
                                      addr_space="Shared").ap()
            z_bounce = nc.dram_tensor("z_bounce", [H2, M], BF16).ap()
            z_all = nc.dram_tensor("z_all", [H2 * NC, M], BF16,
                                   addr_space="Shared").ap()

            # -------- stage 0a: small casts + collective warm-up ----------
            nc.gpsimd.dma_start(x_b[:, :], x_s[:, :])
            nc.gpsimd.dma_start(eps_b[:, :], eps_s[:, :])
            wrm = pers.tile([P, 1], BF16)
            nc.vector.memset(wrm[:], 0.0)
            nc.scalar.dma_start(warm_in[:, :], wrm[:])
            nc.gpsimd.collective_compute(
                "AllGather", mybir.AluOpType.bypass, replica_groups=rg,
                ins=[warm_in.opt()], outs=[warm_out.opt()])

            # W1 / Wms: fp32 loads on scalar queue + DVE cast
            W1f = pers.tile([P, (F_IN // P) * H1], F32)
            nc.scalar.dma_start(
                W1f[:].rearrange("p (t n) -> p t n", n=H1),
                W1.rearrange("(t p) n -> p t n", p=P))
            W1b = pers.tile([P, (F_IN // P) * H1], BF16)
            nc.vector.tensor_copy(W1b[:], W1f[:])
            Wmsf = pers.tile([P, (H1 // P) * (2 * H2)], F32)
            for dt in range(H1 // P):
                nc.scalar.dma_start(Wmsf[:, dt * 256:dt * 256 + H2],
                                    Wm[dt * P:(dt + 1) * P, :])
                nc.scalar.dma_start(Wmsf[:, dt * 256 + H2:dt * 256 + 256],
                                    Ws[dt * P:(dt + 1) * P, :])
            Wms = pers.tile([P, (H1 // P) * (2 * H2)], BF16)
            nc.vector.tensor_copy(Wms[:], Wmsf[:])

            # -------- stage A: xW1 shard + AllGather ------------------
            xT = pers.tile([P, (F_IN // P) * M], BF16)
            for kt in range(F_IN // P):
                nc.sync.dma_start(xT[:, kt * M:(kt + 1) * M],
                                  x_b[:, kt * P:(kt + 1) * P],
                                  transpose=True)
            epsT = pers.tile([P, M], BF16)
            nc.sync.dma_start(epsT[:], eps_b[:, :], transpose=True)

            xw1_loc = pers.tile([P, MT * H1], BF16)
            with tc.tile_pool(name="psA", bufs=2, space="PSUM") as psA:
                for mt in range(MT):
                    pxa = psA.tile([P, H1], F32, tag="pxa")
                    for kt in range(F_IN // P):
                        nc.tensor.matmul(
                            pxa[:],
                            xT[:, kt * M + mt * P: kt * M + (mt + 1) * P],
                            W1b[:, kt * H1:(kt + 1) * H1],
                            start=(kt == 0), stop=(kt == F_IN // P - 1))
                    nc.vector.tensor_copy(
                        xw1_loc[:, mt * H1:(mt + 1) * H1], pxa[:])
            nc.scalar.dma_start(
                xw1_bounce.rearrange("(t p) n -> p t n", p=P),
                xw1_loc[:].rearrange("p (t n) -> p t n", n=H1))
            # AG trigger BEFORE the adj casts: a collective trigger on the
            # gpsimd engine waits for all outstanding SWDGE DMAs, so the
            # casts must be emitted after it.  The casts themselves are
            # row-contiguous halves (bandwidth-bound, minimal descriptors).
            nc.gpsimd.collective_compute(
                "AllGather", mybir.AluOpType.bypass, replica_groups=rg,
                ins=[xw1_bounce.opt()], outs=[xw1_all.opt()])
            for h in range(2):
                nc.gpsimd.dma_start(adj_bh[h][:, :],
                                    adj_s[h * HM:(h + 1) * HM, :])
            xw1_sb = pers.tile([P, KT * H1], BF16)
            nc.scalar.dma_start(
                xw1_sb[:].rearrange("p (t n) -> p t n", n=H1),
                xw1_all.rearrange("(t p) n -> p t n", p=P))

            # -------- stage B: pass 1, hT = relu(xW1^T adjT) ----------
            hT_loc = pers.tile([P, 2 * M], BF16)
            with tc.tile_pool(name="psB", bufs=1, space="PSUM") as psB:
                p1 = [psB.tile([P, 512], F32, tag=f"p1_{i}", name=f"p1_{i}")
                      for i in range(4)]
                for hf in range(2):
                    for kt in range(KT):
                        R = mv.tile([P, HM], BF16, tag="R", name="R", bufs=16)
                        nc.sync.dma_start(
                            R[:], adj_bh[hf][:, kt * P:(kt + 1) * P],
                            transpose=True)
                        nc.scalar.dma_start(
                            adjT[kt * P:(kt + 1) * P, hf * HM:(hf + 1) * HM],
                            R[:])
                        for nt in range(2):
                            nc.tensor.matmul(
                                p1[nt * 2 + hf][:],
                                xw1_sb[:, kt * H1 + nt * P:
                                       kt * H1 + (nt + 1) * P],
                                R[:],
                                start=(kt == 0), stop=(kt == KT - 1))
                for nt in range(2):
                    for hf in range(2):
                        nc.scalar.activation(
                            hT_loc[:, nt * M + hf * 512:
                                   nt * M + (hf + 1) * 512],
                            p1[nt * 2 + hf][:], AF.Relu)

            # -------- stage C: local hWms shard + AllGather -----------
            hwms_loc = pers.tile([P, MT * 2 * H2], BF16)
            with tc.tile_pool(name="psC", bufs=2, space="PSUM") as psC:
                for mt in range(MT):
                    pc = psC.tile([P, 2 * H2], F32, tag="pc")
                    for dt in range(H1 // P):
                        nc.tensor.matmul(
                            pc[:],
                            hT_loc[:, dt * M + mt * P: dt * M + (mt + 1) * P],
                            Wms[:, dt * 256:(dt + 1) * 256],
                            start=(dt == 0), stop=(dt == H1 // P - 1))
                    nc.vector.tensor_copy(
                        hwms_loc[:, mt * 256:(mt + 1) * 256], pc[:])
            nc.scalar.dma_start(
                hwms_bounce.rearrange("(t p) n -> p t n", p=P),
                hwms_loc[:].rearrange("p (t n) -> p t n", n=256))
            nc.gpsimd.collective_compute(
                "AllGather", mybir.AluOpType.bypass, replica_groups=rg,
                ins=[hwms_bounce.opt()], outs=[hwms_all.opt()])
            hwms_sb = pers.tile([P, KT * 256], BF16)
            nc.scalar.dma_start(
                hwms_sb[:].rearrange("p (t n) -> p t n", n=256),
                hwms_all.rearrange("(t p) n -> p t n", p=P))

            # -------- stage D: pass 2, zmT / lsT ----------------------
            zmT = pers.tile([P, M], F32)
            lsT = pers.tile([P, M], F32)
            with tc.tile_pool(name="psD", bufs=1, space="PSUM") as psD:
                p2 = [psD.tile([P, 512], F32, tag=f"p2_{i}", name=f"p2_{i}")
                      for i in range(4)]
                for kt in range(KT):
                    R2 = mv.tile([P, M], BF16, tag="R2", name="R2", bufs=8)
                    nc.sync.dma_start(R2[:], adjT[kt * P:(kt + 1) * P, :])
                    for j in range(2):
                        for hf in range(2):
                            nc.tensor.matmul(
                                p2[j * 2 + hf][:],
                                hwms_sb[:, kt * 256 + j * P:
                                        kt * 256 + (j + 1) * P],
                                R2[:, hf * 512:(hf + 1) * 512],
                                start=(kt == 0), stop=(kt == KT - 1))
                for hf in range(2):
                    nc.vector.tensor_copy(zmT[:, hf * 512:(hf + 1) * 512],
                                          p2[0 * 2 + hf][:])
                    nc.vector.tensor_copy(lsT[:, hf * 512:(hf + 1) * 512],
                                          p2[1 * 2 + hf][:])

            # -------- stage E: z = zm + eps * exp(ls) -----------------
            epsT_f = pers.tile([P, M], F32)
            nc.vector.tensor_copy(epsT_f[:], epsT[:])
            expT = pers.tile([P, M], F32)
            nc.scalar.activation(expT[:], lsT[:], AF.Exp)
            zT = pers.tile([P, M], F32)
            nc.vector.tensor_mul(zT[:], epsT_f[:], expT[:])
            nc.vector.tensor_add(zT[:], zmT[:], zT[:])
            zT_b = pers.tile([P, M], BF16)
            nc.vector.tensor_copy(zT_b[:], zT[:])
            nc.scalar.dma_start(z_bounce[:, :], zT_b[:])
            nc.gpsimd.collective_compute(
                "AllGather", mybir.AluOpType.bypass, replica_groups=rg,
                ins=[z_bounce.opt()], outs=[z_all.opt()])

            # -------- stage F: decoder --------------------------------
            z_sb = pers.tile([P, NC * M], BF16)
            for t in range(NC):
                nc.scalar.dma_start(z_sb[:, t * M:(t + 1) * M],
                                    z_all[t * P:(t + 1) * P, :])
            with tc.tile_pool(name="psF", bufs=6, space="PSUM") as psF:
                i = 0
                for mt in range(MT):
                    for nb in range(NC):
                        for nh in range(2):
                            pd = psF.tile([P, 512], F32, tag="pd")
                            nc.tensor.matmul(
                                pd[:],
                                zT_b[:, mt * P:(mt + 1) * P],
                                z_sb[:, nb * M + nh * 512:
                                     nb * M + (nh + 1) * 512],
                                start=True, stop=True)
                            o_ev = ev.tile([P, 512], F32, tag="o_ev")
                            nc.scalar.activation(o_ev[:], pd[:], AF.Sigmoid)
                            eng = nc.sync if i % 2 == 0 else nc.gpsimd
                            eng.dma_start(
                                out_s[mt * P:(mt + 1) * P,
                                      nb * M + nh * 512:
                                      nb * M + (nh + 1) * 512],
                                o_ev[:])
                            i += 1
    nc.compile()
    return nc


_CACHED = None


def kernel(x, adj, W1, Wm, Ws, eps):
    global _CACHED
    if _CACHED is None:
        _CACHED = build()
    nc = _CACHED
    in_maps = []
    for c in range(NC):
        r0 = c * M
        in_maps.append({
            "x_s": np.ascontiguousarray(x[r0:r0 + M]),
            "adj_s": np.ascontiguousarray(adj[r0:r0 + M]),
            "W1": np.ascontiguousarray(W1),
            "Wm": np.ascontiguousarray(Wm),
            "Ws": np.ascontiguousarray(Ws),
            "eps_s": np.ascontiguousarray(eps[r0:r0 + M]),
        })
    res = run_bass_kernel_spmd(nc, in_maps, core_ids=list(range(NC)))
    out = np.concatenate([res.results[c]["out_s"] for c in range(NC)], axis=0)
    return out.astype(np.float32, copy=False)


if __name__ == "__main__":
    rng = np.random.default_rng(0)
    out = kernel(
        rng.standard_normal((N, F_IN), dtype=np.float32),
        rng.random((N, N), dtype=np.float32),
        (rng.standard_normal((F_IN, H1)) / np.sqrt(F_IN)).astype(np.float32),
        (rng.standard_normal((H1, H2)) / np.sqrt(H1)).astype(np.float32),
        (rng.standard_normal((H1, H2)) / np.sqrt(H1)).astype(np.float32),
        rng.standard_normal((N, H2), dtype=np.float32),
    )
    print(out.shape, np.isnan(out).mean())


# revision 17
# speedup vs baseline: 1.4247x; 1.4247x over previous
"""GCN-VAE forward pass (GCNModelVAE) on 8 Trainium2 NeuronCores.

Row-shards the 8192 nodes across 8 cores (1024 rows each). All big matmuls
run in bf16 on the PE array with fp32 PSUM accumulation; the output is
saturation-dominated (the reference's exp(log_std) overflows), so bf16
operand precision is ample.

Per core (M = 1024 local nodes, P = 128 partitions):
  0. Cast adj_s fp32 -> bf16 as two row-half DRAM tensors (row-contiguous
     SWDGE casts are bandwidth-bound; column-chunked ones are
     descriptor-bound).  x/eps cast contiguously; W1/Wms cast via DVE.
  A. xW1_s = x_s @ W1 via transposed x tiles; AllGather -> xW1_full.
  B. hT_s = relu(xW1_full^T @ adj_s^T): adj^T [128,512] tiles via DMA-xbar
     transpose loads (sync queue), one per (kt, row-half); the transposed
     tiles are also stored to adjT in DRAM (scalar queue) for pass 2.
  C. hWms_s = h_s @ [Wm|Ws] [1024,256] locally; AllGather -> hWms_full.
  D. zmT/lsT = (hWms)^T @ adj_s^T via plain reloads of adjT (no xbar).
  E. zT = zmT + epsT * exp(lsT); AllGather zT -> z_all.
  F. out_s = sigmoid(zT_s^T @ zT_all) [1024,8192] row-block of the decoder.

Queue discipline (avoids HWDGE/SWDGE head-of-line stalls found in traces):
  gpsimd/SWDGE q0: contiguous casts + half the decoder stores
  sync  HWDGE: all xbar transposes, pass-2 loads, half the decoder stores
  scalar HWDGE: bounce stores, gathered loads, adjT stores
"""

import numpy as np

import concourse.bacc as bacc
import concourse.mybir as mybir
import concourse.tile as tile
from concourse.bass_utils import run_bass_kernel_spmd

N = 8192
F_IN = 512
H1 = 256
H2 = 128
NC = 8
M = N // NC          # 1024 rows per core
P = 128
KT = N // P          # 64 k-tiles over the node dimension
MT = M // P          # 8 m-tiles per core
HM = M // 2          # 512 = row-half size
F32 = mybir.dt.float32
BF16 = mybir.dt.bfloat16
AF = mybir.ActivationFunctionType


def build(n_cores=NC):
    nc = bacc.Bacc("TRN2", target_bir_lowering=False, debug=False,
                   num_devices=n_cores)
    x_s = nc.dram_tensor("x_s", [M, F_IN], F32, kind="ExternalInput")
    adj_s = nc.dram_tensor("adj_s", [M, N], F32, kind="ExternalInput")
    W1 = nc.dram_tensor("W1", [F_IN, H1], F32, kind="ExternalInput")
    Wm = nc.dram_tensor("Wm", [H1, H2], F32, kind="ExternalInput")
    Ws = nc.dram_tensor("Ws", [H1, H2], F32, kind="ExternalInput")
    eps_s = nc.dram_tensor("eps_s", [M, H2], F32, kind="ExternalInput")
    out_s = nc.dram_tensor("out_s", [M, N], F32, kind="ExternalOutput")

    rg = [list(range(n_cores))]

    with tile.TileContext(nc) as tc:
        with (
            tc.tile_pool(name="pers", bufs=1) as pers,
            tc.tile_pool(name="mv", bufs=1) as mv,
            tc.tile_pool(name="ev", bufs=6) as ev,
        ):
            # ---------------- DRAM staging (raw internal tensors) --------
            adj_bh = [nc.dram_tensor(f"adj_bh{h}", [HM, N], BF16).ap()
                      for h in range(2)]
            x_b = nc.dram_tensor("x_b", [M, F_IN], BF16).ap()
            eps_b = nc.dram_tensor("eps_b", [M, H2], BF16).ap()
            warm_in = nc.dram_tensor("warm_in", [P, 1], BF16).ap()
            warm_out = nc.dram_tensor("warm_out", [P * NC, 1], BF16,
                                      addr_space="Shared").ap()
            xw1_bounce = nc.dram_tensor("xw1_bounce", [M, H1], BF16).ap()
            xw1_all = nc.dram_tensor("xw1_all", [N, H1], BF16,
                                     addr_space="Shared").ap()
            hwms_bounce = nc.dram_tensor("hwms_bounce", [M, 2 * H2], BF16).ap()
            hwms_all = nc.dram_tensor("hwms_all", [N, 2 * H2], BF16,
                                      addr_space="Shared").ap()
            z_bounce = nc.dram_tensor("z_bounce", [H2, M], BF16).ap()
            z_all = nc.dram_tensor("z_all", [H2 * NC, M], BF16,
                                   addr_space="Shared").ap()

            # -------- stage 0a: small casts + collective warm-up ----------
            nc.gpsimd.dma_start(x_b[:, :], x_s[:, :])
            nc.gpsimd.dma_start(eps_b[:, :], eps_s[:, :])
            wrm = pers.tile([P, 1], BF16)
            nc.vector.memset(wrm[:], 0.0)
            nc.scalar.dma_start(warm_in[:, :], wrm[:])
            nc.gpsimd.collective_compute(
                "AllGather", mybir.AluOpType.bypass, replica_groups=rg,
                ins=[warm_in.opt()], outs=[warm_out.opt()])

            # W1 / Wms: fp32 loads on scalar queue + DVE cast
            W1f = pers.tile([P, (F_IN // P) * H1], F32)
            nc.scalar.dma_start(
                W1f[:].rearrange("p (t n) -> p t n", n=H1),
                W1.rearrange("(t p) n -> p t n", p=P))
            W1b = pers.tile([P, (F_IN // P) * H1], BF16)
            nc.vector.tensor_copy(W1b[:], W1f[:])
            Wmsf = pers.tile([P, (H1 // P) * (2 * H2)], F32)
            for dt in range(H1 // P):
                nc.scalar.dma_start(Wmsf[:, dt * 256:dt * 256 + H2],
                                    Wm[dt * P:(dt + 1) * P, :])
                nc.scalar.dma_start(Wmsf[:, dt * 256 + H2:dt * 256 + 256],
                                    Ws[dt * P:(dt + 1) * P, :])
            Wms = pers.tile([P, (H1 // P) * (2 * H2)], BF16)
            nc.vector.tensor_copy(Wms[:], Wmsf[:])

            # -------- stage A: xW1 shard + AllGather ------------------
            xT = pers.tile([P, (F_IN // P) * M], BF16)
            for kt in range(F_IN // P):
                nc.sync.dma_start(xT[:, kt * M:(kt + 1) * M],
                                  x_b[:, kt * P:(kt + 1) * P],
                                  transpose=True)
            epsT = pers.tile([P, M], BF16)
            nc.sync.dma_start(epsT[:], eps_b[:, :], transpose=True)

            xw1_loc = pers.tile([P, MT * H1], BF16)
            with tc.tile_pool(name="psA", bufs=2, space="PSUM") as psA:
                for mt in range(MT):
                    pxa = psA.tile([P, H1], F32, tag="pxa")
                    for kt in range(F_IN // P):
                        nc.tensor.matmul(
                            pxa[:],
                            xT[:, kt * M + mt * P: kt * M + (mt + 1) * P],
                            W1b[:, kt * H1:(kt + 1) * H1],
                            start=(kt == 0), stop=(kt == F_IN // P - 1))
                    nc.vector.tensor_copy(
                        xw1_loc[:, mt * H1:(mt + 1) * H1], pxa[:])
            nc.scalar.dma_start(
                xw1_bounce.rearrange("(t p) n -> p t n", p=P),
                xw1_loc[:].rearrange("p (t n) -> p t n", n=H1))
            # adj casts: row-contiguous halves (bandwidth-bound, minimal
            # descriptors).  The AG trigger is emitted after them: a
            # collective trigger on gpsimd waits for outstanding SWDGE DMAs
            # anyway, and an AG concurrent with the casts contends for the
            # SDMA engines (measured 175us vs ~35us uncontended).
            for h in range(2):
                nc.gpsimd.dma_start(adj_bh[h][:, :],
                                    adj_s[h * HM:(h + 1) * HM, :])
            nc.gpsimd.collective_compute(
                "AllGather", mybir.AluOpType.bypass, replica_groups=rg,
                ins=[xw1_bounce.opt()], outs=[xw1_all.opt()])
            xw1_sb = pers.tile([P, KT * H1], BF16)
            nc.scalar.dma_start(
                xw1_sb[:].rearrange("p (t n) -> p t n", n=H1),
                xw1_all.rearrange("(t p) n -> p t n", p=P))

            # -------- stage B: pass 1, hT = relu(xW1^T adjT) ----------
            hT_loc = pers.tile([P, 2 * M], BF16)
            with tc.tile_pool(name="psB", bufs=1, space="PSUM") as psB:
                p1 = [psB.tile([P, 512], F32, tag=f"p1_{i}", name=f"p1_{i}")
                      for i in range(4)]
                for hf in range(2):
                    for kt in range(KT):
                        R = mv.tile([P, HM], BF16, tag="R", name="R", bufs=16)
                        nc.sync.dma_start(
                            R[:], adj_bh[hf][:, kt * P:(kt + 1) * P],
                            transpose=True)
                        for nt in range(2):
                            nc.tensor.matmul(
                                p1[nt * 2 + hf][:],
                                xw1_sb[:, kt * H1 + nt * P:
                                       kt * H1 + (nt + 1) * P],
                                R[:],
                                start=(kt == 0), stop=(kt == KT - 1))
                for nt in range(2):
                    for hf in range(2):
                        nc.scalar.activation(
                            hT_loc[:, nt * M + hf * 512:
                                   nt * M + (hf + 1) * 512],
                            p1[nt * 2 + hf][:], AF.Relu)

            # -------- stage C: local hWms shard + AllGather -----------
            hwms_loc = pers.tile([P, MT * 2 * H2], BF16)
            with tc.tile_pool(name="psC", bufs=2, space="PSUM") as psC:
                for mt in range(MT):
                    pc = psC.tile([P, 2 * H2], F32, tag="pc")
                    for dt in range(H1 // P):
                        nc.tensor.matmul(
                            pc[:],
                            hT_loc[:, dt * M + mt * P: dt * M + (mt + 1) * P],
                            Wms[:, dt * 256:(dt + 1) * 256],
                            start=(dt == 0), stop=(dt == H1 // P - 1))
                    nc.vector.tensor_copy(
                        hwms_loc[:, mt * 256:(mt + 1) * 256], pc[:])
            nc.scalar.dma_start(
                hwms_bounce.rearrange("(t p) n -> p t n", p=P),
                hwms_loc[:].rearrange("p (t n) -> p t n", n=256))
            nc.gpsimd.collective_compute(
                "AllGather", mybir.AluOpType.bypass, replica_groups=rg,
                ins=[hwms_bounce.opt()], outs=[hwms_all.opt()])
            hwms_sb = pers.tile([P, KT * 256], BF16)
            nc.scalar.dma_start(
                hwms_sb[:].rearrange("p (t n) -> p t n", n=256),
                hwms_all.rearrange("(t p) n -> p t n", p=P))

            # -------- stage D: pass 2, zmT / lsT ----------------------
            zmT = pers.tile([P, M], F32)
            lsT = pers.tile([P, M], F32)
            with tc.tile_pool(name="psD", bufs=1, space="PSUM") as psD:
                p2 = [psD.tile([P, 512], F32, tag=f"p2_{i}", name=f"p2_{i}")
                      for i in range(4)]
                for hf in range(2):
                    for kt in range(KT):
                        R2 = mv.tile([P, HM], BF16, tag="R", name="R2",
                                     bufs=16)
                        nc.sync.dma_start(
                            R2[:], adj_bh[hf][:, kt * P:(kt + 1) * P],
                            transpose=True)
                        for j in range(2):
                            nc.tensor.matmul(
                                p2[j * 2 + hf][:],
                                hwms_sb[:, kt * 256 + j * P:
                                        kt * 256 + (j + 1) * P],
                                R2[:],
                                start=(kt == 0), stop=(kt == KT - 1))
                for hf in range(2):
                    nc.vector.tensor_copy(zmT[:, hf * 512:(hf + 1) * 512],
                                          p2[0 * 2 + hf][:])
                    nc.vector.tensor_copy(lsT[:, hf * 512:(hf + 1) * 512],
                                          p2[1 * 2 + hf][:])

            # -------- stage E: z = zm + eps * exp(ls) -----------------
            epsT_f = pers.tile([P, M], F32)
            nc.vector.tensor_copy(epsT_f[:], epsT[:])
            expT = pers.tile([P, M], F32)
            nc.scalar.activation(expT[:], lsT[:], AF.Exp)
            zT = pers.tile([P, M], F32)
            nc.vector.tensor_mul(zT[:], epsT_f[:], expT[:])
            nc.vector.tensor_add(zT[:], zmT[:], zT[:])
            zT_b = pers.tile([P, M], BF16)
            nc.vector.tensor_copy(zT_b[:], zT[:])
            nc.scalar.dma_start(z_bounce[:, :], zT_b[:])
            nc.gpsimd.collective_compute(
                "AllGather", mybir.AluOpType.bypass, replica_groups=rg,
                ins=[z_bounce.opt()], outs=[z_all.opt()])

            # -------- stage F: decoder --------------------------------
            z_sb = pers.tile([P, NC * M], BF16)
            for t in range(NC):
                nc.scalar.dma_start(z_sb[:, t * M:(t + 1) * M],
                                    z_all[t * P:(t + 1) * P, :])
            with tc.tile_pool(name="psF", bufs=6, space="PSUM") as psF:
                i = 0
                for mt in range(MT):
                    for nb in range(NC):
                        for nh in range(2):
                            pd = psF.tile([P, 512], F32, tag="pd")
                            nc.tensor.matmul(
                                pd[:],
                                zT_b[:, mt * P:(mt + 1) * P],
                                z_sb[:, nb * M + nh * 512:
                                     nb * M + (nh + 1) * 512],
                                start=True, stop=True)
                            o_ev = ev.tile([P, 512], F32, tag="o_ev")
                            nc.scalar.activation(o_ev[:], pd[:], AF.Sigmoid)
                            eng = nc.sync if i % 2 == 0 else nc.gpsimd
                            eng.dma_start(
                                out_s[mt * P:(mt + 1) * P,
                                      nb * M + nh * 512:
                                      nb * M + (nh + 1) * 512],
                                o_ev[:])
                            i += 1
    nc.compile()
    return nc


_CACHED = None


def kernel(x, adj, W1, Wm, Ws, eps):
    global _CACHED
    if _CACHED is None:
        _CACHED = build()
    nc = _CACHED
    in_maps = []
    for c in range(NC):
        r0 = c * M
        in_maps.append({
            "x_s": np.ascontiguousarray(x[r0:r0 + M]),
            "adj_s": np.ascontiguousarray(adj[r0:r0 + M]),
            "W1": np.ascontiguousarray(W1),
            "Wm": np.ascontiguousarray(Wm),
            "Ws": np.ascontiguousarray(Ws),
            "eps_s": np.ascontiguousarray(eps[r0:r0 + M]),
        })
    res = run_bass_kernel_spmd(nc, in_maps, core_ids=list(range(NC)))
    out = np.concatenate([res.results[c]["out_s"] for c in range(NC)], axis=0)
    return out.astype(np.float32, copy=False)


if __name__ == "__main__":
    rng = np.random.default_rng(0)
    out = kernel(
        rng.standard_normal((N, F_IN), dtype=np.float32),
        rng.random((N, N), dtype=np.float32),
        (rng.standard_normal((F_IN, H1)) / np.sqrt(F_IN)).astype(np.float32),
        (rng.standard_normal((H1, H2)) / np.sqrt(H1)).astype(np.float32),
        (rng.standard_normal((H1, H2)) / np.sqrt(H1)).astype(np.float32),
        rng.standard_normal((N, H2), dtype=np.float32),
    )
    print(out.shape, np.isnan(out).mean())


# revision 18
# speedup vs baseline: 1.6131x; 1.1323x over previous
"""GCN-VAE forward pass (GCNModelVAE) on 8 Trainium2 NeuronCores.

Row-shards the 8192 nodes across 8 cores (1024 rows each). All big matmuls
run in bf16 on the PE array with fp32 PSUM accumulation; the output is
saturation-dominated (the reference's exp(log_std) overflows), so bf16
operand precision is ample.

Per core (M = 1024 local nodes, P = 128 partitions):
  0. Cast adj_s fp32 -> bf16 as two row-half DRAM tensors (row-contiguous
     SWDGE casts are bandwidth-bound; column-chunked ones are
     descriptor-bound).  x/eps cast contiguously; W1/Wms cast via DVE.
  A. xW1_s = x_s @ W1 via transposed x tiles; AllGather -> xW1_full.
  B. hT_s = relu(xW1_full^T @ adj_s^T): adj^T [128,512] tiles via DMA-xbar
     transpose loads (sync queue), one per (kt, row-half); the transposed
     tiles are also stored to adjT in DRAM (scalar queue) for pass 2.
  C. hWms_s = h_s @ [Wm|Ws] [1024,256] locally; AllGather -> hWms_full.
  D. zmT/lsT = (hWms)^T @ adj_s^T via plain reloads of adjT (no xbar).
  E. zT = zmT + epsT * exp(lsT); AllGather zT -> z_all.
  F. out_s = sigmoid(zT_s^T @ zT_all) [1024,8192] row-block of the decoder.

Queue discipline (avoids HWDGE/SWDGE head-of-line stalls found in traces):
  gpsimd/SWDGE q0: contiguous casts + half the decoder stores
  sync  HWDGE: all xbar transposes, pass-2 loads, half the decoder stores
  scalar HWDGE: bounce stores, gathered loads, adjT stores
"""

import numpy as np

import concourse.bacc as bacc
import concourse.mybir as mybir
import concourse.tile as tile
from concourse.bass_utils import run_bass_kernel_spmd

N = 8192
F_IN = 512
H1 = 256
H2 = 128
NC = 8
M = N // NC          # 1024 rows per core
P = 128
KT = N // P          # 64 k-tiles over the node dimension
MT = M // P          # 8 m-tiles per core
HM = M // 2          # 512 = row-half size
F32 = mybir.dt.float32
BF16 = mybir.dt.bfloat16
AF = mybir.ActivationFunctionType


def build(n_cores=NC):
    nc = bacc.Bacc("TRN2", target_bir_lowering=False, debug=False,
                   num_devices=n_cores)
    x_s = nc.dram_tensor("x_s", [M, F_IN], F32, kind="ExternalInput")
    adj_s = nc.dram_tensor("adj_s", [M, N], F32, kind="ExternalInput")
    W1 = nc.dram_tensor("W1", [F_IN, H1], F32, kind="ExternalInput")
    Wm = nc.dram_tensor("Wm", [H1, H2], F32, kind="ExternalInput")
    Ws = nc.dram_tensor("Ws", [H1, H2], F32, kind="ExternalInput")
    eps_s = nc.dram_tensor("eps_s", [M, H2], F32, kind="ExternalInput")
    out_s = nc.dram_tensor("out_s", [M, N], F32, kind="ExternalOutput")

    rg = [list(range(n_cores))]

    with tile.TileContext(nc) as tc:
        with (
            tc.tile_pool(name="pers", bufs=1) as pers,
            tc.tile_pool(name="mv", bufs=1) as mv,
            tc.tile_pool(name="ev", bufs=6) as ev,
        ):
            # ---------------- DRAM staging (raw internal tensors) --------
            adj_b = nc.dram_tensor("adj_b", [M, N], BF16).ap()
            x_b = nc.dram_tensor("x_b", [M, F_IN], BF16).ap()
            eps_b = nc.dram_tensor("eps_b", [M, H2], BF16).ap()
            warm_in = nc.dram_tensor("warm_in", [P, 1], BF16).ap()
            warm_out = nc.dram_tensor("warm_out", [P * NC, 1], BF16,
                                      addr_space="Shared").ap()
            xw1_bounce = nc.dram_tensor("xw1_bounce", [M, H1], BF16).ap()
            xw1_all = nc.dram_tensor("xw1_all", [N, H1], BF16,
                                     addr_space="Shared").ap()
            hwms_bounce = nc.dram_tensor("hwms_bounce", [M, 2 * H2], BF16).ap()
            hwms_all = nc.dram_tensor("hwms_all", [N, 2 * H2], BF16,
                                      addr_space="Shared").ap()
            z_bounce = nc.dram_tensor("z_bounce", [H2, M], BF16).ap()
            z_all = nc.dram_tensor("z_all", [H2 * NC, M], BF16,
                                   addr_space="Shared").ap()

            # -------- stage 0a: small casts + collective warm-up ----------
            nc.gpsimd.dma_start(x_b[:, :], x_s[:, :])
            nc.gpsimd.dma_start(eps_b[:, :], eps_s[:, :])
            wrm = pers.tile([P, 1], BF16)
            nc.vector.memset(wrm[:], 0.0)
            nc.scalar.dma_start(warm_in[:, :], wrm[:])
            nc.gpsimd.collective_compute(
                "AllGather", mybir.AluOpType.bypass, replica_groups=rg,
                ins=[warm_in.opt()], outs=[warm_out.opt()])

            # W1 / Wms: fp32 loads on scalar queue + DVE cast
            W1f = pers.tile([P, (F_IN // P) * H1], F32)
            nc.scalar.dma_start(
                W1f[:].rearrange("p (t n) -> p t n", n=H1),
                W1.rearrange("(t p) n -> p t n", p=P))
            W1b = pers.tile([P, (F_IN // P) * H1], BF16)
            nc.vector.tensor_copy(W1b[:], W1f[:])
            Wmsf = pers.tile([P, (H1 // P) * (2 * H2)], F32)
            for dt in range(H1 // P):
                nc.scalar.dma_start(Wmsf[:, dt * 256:dt * 256 + H2],
                                    Wm[dt * P:(dt + 1) * P, :])
                nc.scalar.dma_start(Wmsf[:, dt * 256 + H2:dt * 256 + 256],
                                    Ws[dt * P:(dt + 1) * P, :])
            Wms = pers.tile([P, (H1 // P) * (2 * H2)], BF16)
            nc.vector.tensor_copy(Wms[:], Wmsf[:])

            # -------- stage A: xW1 shard + AllGather ------------------
            xT = pers.tile([P, (F_IN // P) * M], BF16)
            for kt in range(F_IN // P):
                nc.sync.dma_start(xT[:, kt * M:(kt + 1) * M],
                                  x_b[:, kt * P:(kt + 1) * P],
                                  transpose=True)
            epsT = pers.tile([P, M], BF16)
            nc.sync.dma_start(epsT[:], eps_b[:, :], transpose=True)

            xw1_loc = pers.tile([P, MT * H1], BF16)
            with tc.tile_pool(name="psA", bufs=2, space="PSUM") as psA:
                for mt in range(MT):
                    pxa = psA.tile([P, H1], F32, tag="pxa")
                    for kt in range(F_IN // P):
                        nc.tensor.matmul(
                            pxa[:],
                            xT[:, kt * M + mt * P: kt * M + (mt + 1) * P],
                            W1b[:, kt * H1:(kt + 1) * H1],
                            start=(kt == 0), stop=(kt == F_IN // P - 1))
                    nc.vector.tensor_copy(
                        xw1_loc[:, mt * H1:(mt + 1) * H1], pxa[:])
            nc.scalar.dma_start(
                xw1_bounce.rearrange("(t p) n -> p t n", p=P),
                xw1_loc[:].rearrange("p (t n) -> p t n", n=H1))
            # adj casts: row-contiguous halves (bandwidth-bound, minimal
            # descriptors).  The AG trigger is emitted after them: a
            # collective trigger on gpsimd waits for outstanding SWDGE DMAs
            # anyway, and an AG concurrent with the casts contends for the
            # SDMA engines (measured 175us vs ~35us uncontended).
            for s in range(8):
                nc.gpsimd.dma_start(adj_b[s * P:(s + 1) * P, :],
                                    adj_s[s * P:(s + 1) * P, :])
            nc.gpsimd.collective_compute(
                "AllGather", mybir.AluOpType.bypass, replica_groups=rg,
                ins=[xw1_bounce.opt()], outs=[xw1_all.opt()])
            xw1_sb = pers.tile([P, KT * H1], BF16)
            nc.scalar.dma_start(
                xw1_sb[:].rearrange("p (t n) -> p t n", n=H1),
                xw1_all.rearrange("(t p) n -> p t n", p=P))

            # -------- stage B: pass 1, hT = relu(xW1^T adjT) ----------
            hT_loc = pers.tile([P, 2 * M], BF16)
            with tc.tile_pool(name="psB", bufs=1, space="PSUM") as psB:
                p1 = [psB.tile([P, 512], F32, tag=f"p1_{i}", name=f"p1_{i}")
                      for i in range(4)]
                for kt in range(KT):
                    R = mv.tile([P, M], BF16, tag="R", name="R", bufs=16)
                    nc.sync.dma_start(R[:], adj_b[:, kt * P:(kt + 1) * P],
                                      transpose=True)
                    for nt in range(2):
                        for hf in range(2):
                            nc.tensor.matmul(
                                p1[nt * 2 + hf][:],
                                xw1_sb[:, kt * H1 + nt * P:
                                       kt * H1 + (nt + 1) * P],
                                R[:, hf * 512:(hf + 1) * 512],
                                start=(kt == 0), stop=(kt == KT - 1))
                for nt in range(2):
                    for hf in range(2):
                        nc.scalar.activation(
                            hT_loc[:, nt * M + hf * 512:
                                   nt * M + (hf + 1) * 512],
                            p1[nt * 2 + hf][:], AF.Relu)

            # -------- stage C: local hWms shard + AllGather -----------
            hwms_loc = pers.tile([P, MT * 2 * H2], BF16)
            with tc.tile_pool(name="psC", bufs=2, space="PSUM") as psC:
                for mt in range(MT):
                    pc = psC.tile([P, 2 * H2], F32, tag="pc")
                    for dt in range(H1 // P):
                        nc.tensor.matmul(
                            pc[:],
                            hT_loc[:, dt * M + mt * P: dt * M + (mt + 1) * P],
                            Wms[:, dt * 256:(dt + 1) * 256],
                            start=(dt == 0), stop=(dt == H1 // P - 1))
                    nc.vector.tensor_copy(
                        hwms_loc[:, mt * 256:(mt + 1) * 256], pc[:])
            nc.scalar.dma_start(
                hwms_bounce.rearrange("(t p) n -> p t n", p=P),
                hwms_loc[:].rearrange("p (t n) -> p t n", n=256))
            nc.gpsimd.collective_compute(
                "AllGather", mybir.AluOpType.bypass, replica_groups=rg,
                ins=[hwms_bounce.opt()], outs=[hwms_all.opt()])
            hwms_sb = pers.tile([P, KT * 256], BF16)
            nc.scalar.dma_start(
                hwms_sb[:].rearrange("p (t n) -> p t n", n=256),
                hwms_all.rearrange("(t p) n -> p t n", p=P))

            # -------- stage D: pass 2, zmT / lsT ----------------------
            zmT = pers.tile([P, M], F32)
            lsT = pers.tile([P, M], F32)
            with tc.tile_pool(name="psD", bufs=1, space="PSUM") as psD:
                p2 = [psD.tile([P, 512], F32, tag=f"p2_{i}", name=f"p2_{i}")
                      for i in range(4)]
                for kt in range(KT):
                    R2 = mv.tile([P, M], BF16, tag="R", name="R2", bufs=16)
                    nc.sync.dma_start(R2[:], adj_b[:, kt * P:(kt + 1) * P],
                                      transpose=True)
                    for j in range(2):
                        for hf in range(2):
                            nc.tensor.matmul(
                                p2[j * 2 + hf][:],
                                hwms_sb[:, kt * 256 + j * P:
                                        kt * 256 + (j + 1) * P],
                                R2[:, hf * 512:(hf + 1) * 512],
                                start=(kt == 0), stop=(kt == KT - 1))
                for hf in range(2):
                    nc.vector.tensor_copy(zmT[:, hf * 512:(hf + 1) * 512],
                                          p2[0 * 2 + hf][:])
                    nc.vector.tensor_copy(lsT[:, hf * 512:(hf + 1) * 512],
                                          p2[1 * 2 + hf][:])

            # -------- stage E: z = zm + eps * exp(ls) -----------------
            epsT_f = pers.tile([P, M], F32)
            nc.vector.tensor_copy(epsT_f[:], epsT[:])
            expT = pers.tile([P, M], F32)
            nc.scalar.activation(expT[:], lsT[:], AF.Exp)
            zT = pers.tile([P, M], F32)
            nc.vector.tensor_mul(zT[:], epsT_f[:], expT[:])
            nc.vector.tensor_add(zT[:], zmT[:], zT[:])
            zT_b = pers.tile([P, M], BF16)
            nc.vector.tensor_copy(zT_b[:], zT[:])
            nc.scalar.dma_start(z_bounce[:, :], zT_b[:])
            nc.gpsimd.collective_compute(
                "AllGather", mybir.AluOpType.bypass, replica_groups=rg,
                ins=[z_bounce.opt()], outs=[z_all.opt()])

            # -------- stage F: decoder --------------------------------
            z_sb = pers.tile([P, NC * M], BF16)
            for t in range(NC):
                nc.scalar.dma_start(z_sb[:, t * M:(t + 1) * M],
                                    z_all[t * P:(t + 1) * P, :])
            with tc.tile_pool(name="psF", bufs=6, space="PSUM") as psF:
                i = 0
                for mt in range(MT):
                    for nb in range(NC):
                        for nh in range(2):
                            pd = psF.tile([P, 512], F32, tag="pd")
                            nc.tensor.matmul(
                                pd[:],
                                zT_b[:, mt * P:(mt + 1) * P],
                                z_sb[:, nb * M + nh * 512:
                                     nb * M + (nh + 1) * 512],
                                start=True, stop=True)
                            o_ev = ev.tile([P, 512], F32, tag="o_ev")
                            nc.scalar.activation(o_ev[:], pd[:], AF.Sigmoid)
                            eng = nc.sync if i % 2 == 0 else nc.gpsimd
                            eng.dma_start(
                                out_s[mt * P:(mt + 1) * P,
                                      nb * M + nh * 512:
                                      nb * M + (nh + 1) * 512],
                                o_ev[:])
                            i += 1
    nc.compile()
    return nc


_CACHED = None


def kernel(x, adj, W1, Wm, Ws, eps):
    global _CACHED
    if _CACHED is None:
        _CACHED = build()
    nc = _CACHED
    in_maps = []
    for c in range(NC):
        r0 = c * M
        in_maps.append({
            "x_s": np.ascontiguousarray(x[r0:r0 + M]),
            "adj_s": np.ascontiguousarray(adj[r0:r0 + M]),
            "W1": np.ascontiguousarray(W1),
            "Wm": np.ascontiguousarray(Wm),
            "Ws": np.ascontiguousarray(Ws),
            "eps_s": np.ascontiguousarray(eps[r0:r0 + M]),
        })
    res = run_bass_kernel_spmd(nc, in_maps, core_ids=list(range(NC)))
    out = np.concatenate([res.results[c]["out_s"] for c in range(NC)], axis=0)
    return out.astype(np.float32, copy=False)


if __name__ == "__main__":
    rng = np.random.default_rng(0)
    out = kernel(
        rng.standard_normal((N, F_IN), dtype=np.float32),
        rng.random((N, N), dtype=np.float32),
        (rng.standard_normal((F_IN, H1)) / np.sqrt(F_IN)).astype(np.float32),
        (rng.standard_normal((H1, H2)) / np.sqrt(H1)).astype(np.float32),
        (rng.standard_normal((H1, H2)) / np.sqrt(H1)).astype(np.float32),
        rng.standard_normal((N, H2), dtype=np.float32),
    )
    print(out.shape, np.isnan(out).mean())
